# revision 20
# baseline (speedup 1.0000x reference)
"""Trainium2 Bass kernel for nn_Block_19301583028789.

Pipeline (per batch element): channel-mixing Linear -> erf-GELU -> S4D (FFT conv
in the reference; computed here as an exact chunked linear recurrence) -> FiLM
-> erf-GELU -> per-channel residual.

Sharding: data-parallel over batch B=16 across 8 cores (2 batches/core);
all parameters replicated.

S4D math: y = u * K + D*u with K[m] = 2 Re sum_n coef_n lam_n^m.  Split L into
C=128 chunks of T=128.  Per chunk: local causal conv = lower-tri Toeplitz
matmul; cross-chunk carry = rank-N apply of the complex mode state
S[n,c] = sum_{c'<=c} mu^{c-c'} Z[n,c'], Z = per-chunk Vandermonde summary
(matmul), mu = lam^T.  The state scan runs hierarchically on DVE
(radix-8 within-group, sequential across 16 groups, then combine).
"""

import numpy as np

import concourse.bass as bass
import concourse.tile as tile
import concourse.mybir as mybir
from concourse.bass_utils import run_bass_kernel_spmd

B, H, L = 16, 64, 16384
N, CD = 64, 32
T = 128
C = L // T          # 128 chunks
G1 = 8              # chunks per group (L1 radix)
NG = C // G1        # 16 groups
NCORES = 8
BLOC = B // NCORES  # 2
FP32 = mybir.dt.float32
BF16 = mybir.dt.bfloat16
AF = mybir.ActivationFunctionType

_CACHE = {}


def _split_tail_drain_waits(nc, max_waits=1):
    """Walrus TPB_CTRL lowering only accepts 1 sync-wait per Drain/NoOp; Tile's
    tail drain accumulates one wait per outstanding proc.  Hoist extras onto
    NoOps inserted right before the offending instruction."""
    for fn in nc.m.functions:
        for blk in fn.blocks:
            insts = blk.instructions
            i = 0
            while i < len(insts):
                inst = insts[i]
                si = inst.sync_info
                if si is not None and len(si.on_wait) > max_waits:
                    extra = list(si.on_wait[:-max_waits])
                    keep = list(si.on_wait[-max_waits:])
                    nops = [
                        mybir.InstNoOp(
                            name=f"{inst.name}-waitsplit{k}",
                            opcode="NoOp",
                            engine=inst.engine,
                            sync_info=mybir.SyncInfo(on_wait=[w], on_update=[]),
                        )
                        for k, w in enumerate(extra)
                    ]
                    si.on_wait = keep
                    for k, nop in enumerate(nops):
                        insts.insert(i + k, nop)
                    i += len(nops)
                i += 1


def _host_params(log_dt, log_A_real, A_imag, C_re, C_im, D, W_lin, b_lin):
    """Parameter-derived small constant matrices (fp64 host math)."""
    dt = np.exp(log_dt.astype(np.float64))[:, None]            # [H,1]
    A = -np.exp(log_A_real.astype(np.float64)) + 1j * A_imag.astype(np.float64)
    dtA = A * dt                                               # [H,N]
    coef = (C_re.astype(np.float64) + 1j * C_im.astype(np.float64)) \
        * (np.exp(dtA) - 1.0) / A                              # [H,N]

    # lam^k = exp(dtA*k), k = 0..T+1
    ks = np.arange(T + 2)
    lp = np.exp(dtA[:, :, None] * ks[None, None, :])           # [H,N,T+2]

    # K kernel first T taps; fold D into tap 0
    K = 2.0 * np.real(np.einsum("hn,hnm->hm", coef, lp[:, :, :T]))  # [H,T]
    K[:, 0] += D.astype(np.float64)

    # Toeplitz lhsT[j,t] = K[t-j] (t>=j)
    idx = np.arange(T)
    tm = idx[None, :] - idx[:, None]                           # [j,t]
    mask = tm >= 0
    Ktoep = np.where(mask, K[:, np.clip(tm, 0, T - 1)], 0.0)   # [H,j,t]

    # Z summary lhsT[t,n] = lam^(T-1-t)
    Alq = lp[:, :, ::-1][:, :, 2:T + 2]                        # lam^(T-1-t): [H,N,T] t-minor
    A_re = np.transpose(Alq.real, (0, 2, 1))                   # [H,T,N]
    A_im = np.transpose(Alq.imag, (0, 2, 1))

    # carry apply P[t,n] = 2*coef*lam^(t+1); lhsT [n,t]; im-part negated
    P = 2.0 * coef[:, :, None] * lp[:, :, 1:T + 1]             # [H,N,T]
    P_re = P.real                                              # [H,N,T] already [n,t]
    P_imn = -P.imag

    mu = lp[:, :, T]                                           # lam^T [H,N]
    nu = np.exp(dtA * T * G1)                                  # mu^G1
    comb = np.exp(dtA[:, :, None] * (T * (np.arange(1, G1 + 1))[None, None, :]))
    # comb[..,k] = mu^(k+1), k=0..G1-1

    f32 = lambda a: np.ascontiguousarray(a, dtype=np.float32)
    bf = lambda a: np.ascontiguousarray(a.astype(np.float32), dtype=np.float32)
    # scan coefficient tiles: [N(part), H] layout
    return {
        "Ktoep": f32(Ktoep),
        "A_re": f32(A_re), "A_im": f32(A_im),
        "P_re": bf(P_re), "P_imn": bf(P_imn),
        "mu_re": f32(mu.real.T), "mu_im": f32(mu.imag.T),          # [N,H]
        "nu_re": f32(nu.real.T), "nu_im": f32(nu.imag.T),
        "comb_re": f32(np.transpose(comb.real, (1, 2, 0))),        # [N,G1,H]
        "comb_im": f32(np.transpose(comb.imag, (1, 2, 0))),
        "WB": f32(np.concatenate([W_lin.T, b_lin[None, :]], 0)),   # [H+1,H]
    }


def _build(scan_dtype=BF16):
    nc = bass.Bass("TRN2", target_bir_lowering=False, debug=False)

    dram = {}
    def din(name, shape, dtype=FP32):
        dram[name] = nc.dram_tensor(name, list(shape), dtype, kind="ExternalInput")
        return dram[name]

    x_in = din("x_loc", [BLOC, H, L])
    WB = din("WB", [H + 1, H])
    Ktoep = din("Ktoep", [H, T, T])
    A_re = din("A_re", [H, T, N]); A_im = din("A_im", [H, T, N])
    P_re = din("P_re", [H, N, T], BF16); P_imn = din("P_imn", [H, N, T], BF16)
    mu_re = din("mu_re", [N, H], BF16); mu_im = din("mu_im", [N, H], BF16)
    nu_re = din("nu_re", [N, H], BF16); nu_im = din("nu_im", [N, H], BF16)
    comb_re = din("comb_re", [N, G1, H], BF16); comb_im = din("comb_im", [N, G1, H], BF16)
    eye = din("eye", [128, 128])
    film_WT = din("film_WT", [CD, 2 * H])
    film_bl = din("film_bl", [1, 2 * H])
    condT = din("condT", [CD, BLOC])
    ones1 = din("ones1", [1, BLOC])
    res_w_row = din("res_w_row", [1, H])
    gb_scratch = nc.dram_tensor("gb_scratch", [2 * H * BLOC], FP32)
    y_out = nc.dram_tensor("y_out", [BLOC, H, L], FP32, kind="ExternalOutput")

    xv = x_in.ap().rearrange("b h (c t) -> b h c t", t=T)
    yv = y_out.ap().rearrange("b h (c t) -> b h c t", t=T)

    with tile.TileContext(nc) as tc:
        with (
            tc.tile_pool(name="big", bufs=1) as big,
            tc.tile_pool(name="xhl", bufs=3) as xhl,
            tc.tile_pool(name="par", bufs=3) as par,
            tc.tile_pool(name="ev", bufs=3) as ev,
            tc.tile_pool(name="tmp", bufs=1) as tmp,
            tc.tile_pool(name="outp", bufs=3) as outp,
            tc.tile_pool(name="cst", bufs=1) as cst,
            tc.tile_pool(name="ps_w", bufs=2, space="PSUM") as ps_w,
            tc.tile_pool(name="ps_z", bufs=3, space="PSUM") as ps_z,
            tc.tile_pool(name="ps_y", bufs=2, space="PSUM") as ps_y,
            tc.tile_pool(name="ps_t", bufs=1, space="PSUM") as ps_t,
        ):
            # ---- resident tensors ----
            u = big.tile([128, H * BLOC * C], FP32, tag="u")      # [t,(h,b,c)]
            uv = u[:].rearrange("t (h b c) -> t h b c", h=H, b=BLOC)
            uc = u[:].rearrange("t (h b c) -> t b c h", h=H, b=BLOC)
            # scan state, re/im interleaved: [n, (h,b,g,c_rel,comp)]
            S = big.tile([N, H * BLOC * C * 2], scan_dtype, tag="S")
            Sv = S[:].rearrange("n (h b g r p) -> n h b g r p", h=H, b=BLOC, g=NG, r=G1)

            eye_sb = cst.tile([128, 128], FP32, tag="eye")
            nc.sync.dma_start(eye_sb[:], eye.ap())
            wb_sb = cst.tile([H + 1, H], FP32, tag="wb")
            nc.sync.dma_start(wb_sb[:], WB.ap())
            muT = cst.tile([N, 2 * H], BF16, tag="mu")
            nc.sync.dma_start(muT[:, 0:H], mu_re.ap())
            nc.sync.dma_start(muT[:, H:2 * H], mu_im.ap())
            nuT = cst.tile([N, 2 * H], BF16, tag="nu")
            nc.sync.dma_start(nuT[:, 0:H], nu_re.ap())
            nc.sync.dma_start(nuT[:, H:2 * H], nu_im.ap())
            combT = cst.tile([N, 2 * G1 * H], BF16, tag="comb")
            nc.sync.dma_start(
                combT[:, 0:G1 * H].rearrange("n (r h) -> n r h", r=G1), comb_re.ap())
            nc.sync.dma_start(
                combT[:, G1 * H:].rearrange("n (r h) -> n r h", r=G1), comb_im.ap())
            rwB = cst.tile([128, H], FP32, tag="rw")
            nc.sync.dma_start(rwB[:], res_w_row.ap().broadcast_to([128, H]))

            # ---- FiLM prologue: gb = film_W @ cond + film_b, broadcast ----
            fwt_sb = cst.tile([CD, 2 * H], FP32, tag="fwt")
            nc.sync.dma_start(fwt_sb[:], film_WT.ap())
            fbl_sb = cst.tile([1, 2 * H], FP32, tag="fbl")
            nc.sync.dma_start(fbl_sb[:], film_bl.ap())
            ct_sb = cst.tile([CD, BLOC], FP32, tag="ct")
            nc.sync.dma_start(ct_sb[:], condT.ap())
            on_sb = cst.tile([1, BLOC], FP32, tag="on")
            nc.sync.dma_start(on_sb[:], ones1.ap())
            gps = ps_z.tile([2 * H, BLOC], FP32, tag="z")
            nc.tensor.matmul(gps[:], fwt_sb[:], ct_sb[:], start=True, stop=False)
            nc.tensor.matmul(gps[:], fbl_sb[:], on_sb[:], start=False, stop=True)
            gb_sb = cst.tile([2 * H, BLOC], FP32, tag="gb")
            nc.scalar.copy(gb_sb[:], gps[:])
            nc.sync.dma_start(gb_scratch.ap().rearrange("(r b) -> r b", b=BLOC), gb_sb[:])
            gbB = cst.tile([128, 2 * H * BLOC], FP32, tag="gbB")
            nc.sync.dma_start(
                gbB[:], gb_scratch.ap().rearrange("f -> f")[None, :].broadcast_to(
                    [128, 2 * H * BLOC]))

            # ---- phase A: u = gelu(W x + b), transposed to [t,(h,b,c)] ----
            XC = 4  # c-tiles per x DMA
            for b in range(BLOC):
                for c0 in range(0, C, XC):
                    xt = xhl.tile([H + 1, XC * T], FP32, tag="xt")
                    nc.sync.dma_start(
                        xt[0:H, :].rearrange("h (c t) -> h c t", c=XC),
                        xv[b, :, c0:c0 + XC, :])
                    nc.vector.memset(xt[H:H + 1, :], 1.0)
                    for cc in range(0, XC, 2):
                        c = c0 + cc
                        wp = ps_w.tile([T, 2 * H], FP32)
                        nc.tensor.matmul(
                            wp[:, 0:H], xt[:, cc * T:(cc + 1) * T], wb_sb[:],
                            start=True, stop=True, skip_group_check=True)
                        nc.tensor.matmul(
                            wp[:, H:2 * H], xt[:, (cc + 1) * T:(cc + 2) * T], wb_sb[:],
                            start=True, stop=True, skip_group_check=True)
                        wpv = wp[:].rearrange("t (c h) -> t c h", c=2)
                        ucv = uc[:, b, c:c + 2, :]
                        nc.scalar.activation(ucv, wpv, AF.Gelu)

            # ---- phases B/C/D/E pipelined over two h-halves: the DVE scan of
            # one half overlaps the PE/ACT work of the other ----
            mre = muT[:, 0:H]; mim = muT[:, H:2 * H]
            nre = nuT[:, 0:H]; nim = nuT[:, H:2 * H]
            cv = combT[:].rearrange("n (p r h) -> n p r h", p=2, r=G1)

            def cfma(dst_re, dst_im, w_re, w_im, s_re, s_im, fshape):
                # dst += w * s (complex), w broadcast along trailing dims
                t1 = tmp.tile([N, 2048], BF16, tag="t1")
                t2 = tmp.tile([N, 2048], BF16, tag="t2")
                nf = int(np.prod(fshape))
                a = t1[:, :nf].rearrange("n (x y z) -> n x y z", x=fshape[0], y=fshape[1])
                bq = t2[:, :nf].rearrange("n (x y z) -> n x y z", x=fshape[0], y=fshape[1])
                wr = w_re.broadcast_to([N, *fshape])
                wi = w_im.broadcast_to([N, *fshape])
                nc.vector.tensor_mul(a, wr, s_re)
                nc.vector.tensor_mul(bq, wi, s_im)
                nc.vector.tensor_sub(a, a, bq)
                nc.vector.tensor_add(dst_re, dst_re, a)
                nc.vector.tensor_mul(a, wr, s_im)
                nc.vector.tensor_mul(bq, wi, s_re)
                nc.vector.tensor_add(a, a, bq)
                nc.vector.tensor_add(dst_im, dst_im, a)

            NSPLIT = 4
            HH = H // NSPLIT
            Sflat = S[:].rearrange("n (h b c p) -> n h b c p", h=H, b=BLOC, c=C)
            for half in range(NSPLIT):
                h0, h1 = half * HH, (half + 1) * HH

                # -- B: Z summaries for this half --
                for h in range(h0, h1):
                    are = par.tile([T, N], FP32, tag="are")
                    nc.sync.dma_start(are[:], A_re.ap()[h])
                    aim = par.tile([T, N], FP32, tag="aim")
                    nc.sync.dma_start(aim[:], A_im.ap()[h])
                    rhs = uv[:, h, :, :]  # [t,(b,c)] 3D ap
                    zr = ps_z.tile([N, BLOC * C], FP32, tag="z")
                    nc.tensor.matmul(zr[:], are[:], rhs, start=True, stop=True)
                    zi = ps_z.tile([N, BLOC * C], FP32, tag="z")
                    nc.tensor.matmul(zi[:], aim[:], rhs, start=True, stop=True)
                    zrv = zr[:].rearrange("n (b c) -> n b c", b=BLOC)
                    ziv = zi[:].rearrange("n (b c) -> n b c", b=BLOC)
                    for b in range(BLOC):
                        nc.scalar.copy(
                            Sv[:, h, b, :, :, 0].rearrange("n g r -> n (g r)"), zrv[:, b, :])
                        nc.scalar.copy(
                            Sv[:, h, b, :, :, 1].rearrange("n g r -> n (g r)"), ziv[:, b, :])

                # -- C: hierarchical scan for this half (DVE only) --
                hs = slice(h0, h1)
                for r in range(1, G1):
                    cfma(Sv[:, hs, :, :, r, 0], Sv[:, hs, :, :, r, 1],
                         mre[:, hs, None, None], mim[:, hs, None, None],
                         Sv[:, hs, :, :, r - 1, 0], Sv[:, hs, :, :, r - 1, 1],
                         (HH, BLOC, NG))
                for g in range(1, NG):
                    cfma(Sv[:, hs, :, g, G1 - 1, 0], Sv[:, hs, :, g, G1 - 1, 1],
                         nre[:, hs, None], nim[:, hs, None],
                         Sv[:, hs, :, g - 1, G1 - 1, 0], Sv[:, hs, :, g - 1, G1 - 1, 1],
                         (HH, BLOC, 1))
                for r in range(G1 - 1):
                    cfma(Sv[:, hs, :, 1:NG, r, 0], Sv[:, hs, :, 1:NG, r, 1],
                         cv[:, 0, r, hs, None, None], cv[:, 1, r, hs, None, None],
                         Sv[:, hs, :, 0:NG - 1, G1 - 1, 0], Sv[:, hs, :, 0:NG - 1, G1 - 1, 1],
                         (HH, BLOC, NG - 1))

                # -- D/E: toeplitz + carry apply + output assembly for this half --
                for h in range(h0, h1):
                    kt = par.tile([T, T], FP32, tag="kt")
                    nc.sync.dma_start(kt[:], Ktoep.ap()[h])
                    pre = par.tile([N, T], BF16, tag="pre")
                    nc.sync.dma_start(pre[:], P_re.ap()[h])
                    pim = par.tile([N, T], BF16, tag="pim")
                    nc.sync.dma_start(pim[:], P_imn.ap()[h])
                    yp = ps_y.tile([T, BLOC * C], FP32)
                    ypv = yp[:].rearrange("t (b c) -> t b c", b=BLOC)
                    nc.tensor.matmul(yp[:], kt[:], uv[:, h, :, :],
                                     start=True, stop=False)
                    nc.tensor.matmul(ypv[:, :, 1:C], pre[:],
                                     Sflat[:, h, :, 0:C - 1, 0],
                                     start=False, stop=False)
                    nc.tensor.matmul(ypv[:, :, 1:C], pim[:],
                                     Sflat[:, h, :, 0:C - 1, 1],
                                     start=False, stop=True)
                    z1 = ev.tile([T, BLOC * C], FP32, tag="z1")
                    nc.scalar.copy(z1[:], yp[:])
                    z1v = z1[:].rearrange("t (b c) -> t b c", b=BLOC)
                    for b in range(BLOC):
                        tp = ps_t.tile([128, 128], FP32)
                        nc.tensor.transpose(tp[:], z1v[:, b, :], eye_sb[:])
                        yt = outp.tile([128, T], FP32, tag="yt")
                        nc.scalar.activation(
                            yt[:], tp[:], AF.Gelu,
                            bias=gbB[:, (H + h) * BLOC + b:(H + h) * BLOC + b + 1],
                            scale=gbB[:, h * BLOC + b:h * BLOC + b + 1])
                        xc = outp.tile([128, T], FP32, tag="xc")
                        nc.sync.dma_start(xc[:], xv[b, h, :, :])
                        nc.vector.tensor_scalar_mul(xc[:], xc[:], rwB[:, h:h + 1])
                        nc.vector.tensor_add(yt[:], yt[:], xc[:])
                        nc.sync.dma_start(yv[b, h, :, :], yt[:])

    _split_tail_drain_waits(nc)
    return nc


def kernel(**inputs):
    key = "k"
    if key not in _CACHE:
        _CACHE[key] = _build()
    nc = _CACHE[key]

    hp = _host_params(
        inputs["log_dt"], inputs["log_A_real"], inputs["A_imag"],
        inputs["C_re"], inputs["C_im"], inputs["D"],
        inputs["W_lin"], inputs["b_lin"])

    x = np.ascontiguousarray(inputs["x"], dtype=np.float32)
    cond = np.ascontiguousarray(inputs["conditional_information"], dtype=np.float32)
    film_W = np.ascontiguousarray(inputs["film_W"], dtype=np.float32)
    film_b = np.ascontiguousarray(inputs["film_b"], dtype=np.float32)
    res_w = np.ascontiguousarray(inputs["res_w"], dtype=np.float32)

    bf = lambda a: np.ascontiguousarray(a, dtype=np.float32).astype(
        np.dtype("bfloat16") if False else np.float32)
    import ml_dtypes
    tobf = lambda a: np.ascontiguousarray(a.astype(ml_dtypes.bfloat16))

    common = {
        "WB": hp["WB"], "Ktoep": hp["Ktoep"],
        "A_re": hp["A_re"], "A_im": hp["A_im"],
        "P_re": tobf(hp["P_re"]), "P_imn": tobf(hp["P_imn"]),
        "mu_re": tobf(hp["mu_re"]), "mu_im": tobf(hp["mu_im"]),
        "nu_re": tobf(hp["nu_re"]), "nu_im": tobf(hp["nu_im"]),
        "comb_re": tobf(hp["comb_re"]), "comb_im": tobf(hp["comb_im"]),
        "eye": np.eye(128, dtype=np.float32),
        "film_WT": np.ascontiguousarray(film_W.T),
        "film_bl": film_b[None, :],
        "ones1": np.ones((1, BLOC), np.float32),
        "res_w_row": res_w[None, :],
    }
    in_maps = []
    for c_ in range(NCORES):
        m = dict(common)
        m["x_loc"] = np.ascontiguousarray(x[c_ * BLOC:(c_ + 1) * BLOC])
        m["condT"] = np.ascontiguousarray(cond[c_ * BLOC:(c_ + 1) * BLOC].T)
        in_maps.append(m)

    res = run_bass_kernel_spmd(nc, in_maps, core_ids=list(range(NCORES)))
    out = np.concatenate([res.results[c_]["y_out"] for c_ in range(NCORES)], axis=0)
    return out.astype(np.float32)
